# revision 30
# baseline (speedup 1.0000x reference)
"""Trainium2 Bass kernel for the e3nn depthwise (uvu) tensor product.

Per edge e (E=50000), channel u (64):
    out[e, u, (k,k3)] = w[e,k,u] * sum_{i1,j2} x1[e,u,(l1,i1)] * x2[e,(l2,j2)]
                        * alpha_k * w3j_k[i1,j2,k3]

Strategy (all fp32, edge dim on SBUF partitions, fully contiguous DMAs):
  1. PE builds per-edge scalars T[e, c] = sum_j x2[e,j]*bmat[j,c] for the 129
     nonzero (k,k3,i1) combinations (bmat baked on host, alpha folded in).
  2. DVE/GPSIMD compute each output column group (k,k3) as a chain of
     fused multiply-accumulate ops: acc = x1_slice * T_col (+ acc), where the
     T column is a per-partition scalar. Then one broadcast multiply by the
     per-(e,k,u) weight per instruction k.
Sharded pure data-parallel over edges across 8 NeuronCores.
"""

import math
from contextlib import ExitStack

import numpy as np

# ---------------- problem constants (hardcoded) ----------------
E_TOTAL = 50000
N_CORES = 8
E_PER_CORE = E_TOTAL // N_CORES  # 6250
P = 128  # edges per tile (SBUF partitions)

L_MAX = 2
IRREPS_IN1 = [(64, 0, 1), (64, 1, -1), (64, 2, 1)]
IRREPS_IN2 = [(1, 0, 1), (1, 1, -1), (1, 2, 1)]

IN1_DIM = 576
IN2_DIM = 9
W_DIM = 960
OUT_DIM = 3264


def _su2_cg(j1, j2, j3):
    f = math.factorial
    C = np.zeros((2 * j1 + 1, 2 * j2 + 1, 2 * j3 + 1))
    for m1 in range(-j1, j1 + 1):
        for m2 in range(-j2, j2 + 1):
            m3 = m1 + m2
            if abs(m3) > j3:
                continue
            pref = math.sqrt((2 * j3 + 1) * f(j3 + j1 - j2) * f(j3 - j1 + j2) * f(j1 + j2 - j3) / f(j1 + j2 + j3 + 1))
            pref *= math.sqrt(f(j3 + m3) * f(j3 - m3) * f(j1 - m1) * f(j1 + m1) * f(j2 - m2) * f(j2 + m2))
            s = 0.0
            for k in range(j1 + j2 - j3 + 1):
                t = [k, j1 + j2 - j3 - k, j1 - m1 - k, j2 + m2 - k, j3 - j2 + m1 + k, j3 - j1 - m2 + k]
                if min(t) < 0:
                    continue
                s += (-1) ** k / (f(t[0]) * f(t[1]) * f(t[2]) * f(t[3]) * f(t[4]) * f(t[5]))
            C[j1 + m1, j2 + m2, j3 + m3] = pref * s
    return C / math.sqrt(2 * j3 + 1)


def _q(l):
    q = np.zeros((2 * l + 1, 2 * l + 1), dtype=np.complex128)
    for m in range(-l, 0):
        q[l + m, l + abs(m)] = 1 / math.sqrt(2)
        q[l + m, l - abs(m)] = -1j / math.sqrt(2)
    q[l, l] = 1.0
    for m in range(1, l + 1):
        q[l + m, l + abs(m)] = (-1) ** m / math.sqrt(2)
        q[l + m, l - abs(m)] = 1j * (-1) ** m / math.sqrt(2)
    return (-1j) ** l * q


def _w3j(l1, l2, l3):
    C = np.einsum("ij,kl,mn,ikm->jln", _q(l1), _q(l2), np.conj(_q(l3)), _su2_cg(l1, l2, l3))
    return np.real(C).astype(np.float64)


def _build_instr():
    irreps_out, instr = [], []
    for i, (mul1, l1, p1) in enumerate(IRREPS_IN1):
        for j, (mul2, l2, p2) in enumerate(IRREPS_IN2):
            for l3 in range(abs(l1 - l2), l1 + l2 + 1):
                if l3 <= L_MAX:
                    irreps_out.append((mul1, l3, p1 * p2))
                    instr.append((i, j, len(irreps_out) - 1))
    key = lambda k: (irreps_out[k][1], -irreps_out[k][2] * (-1) ** irreps_out[k][1])
    inv = sorted(range(len(irreps_out)), key=key)
    p = {old: new for new, old in enumerate(inv)}
    irreps_sorted = [irreps_out[i] for i in inv]
    instr = sorted([(i, j, p[k]) for (i, j, k) in instr], key=lambda t: t[2])
    return irreps_sorted, instr


IRREPS_OUT, INSTR = _build_instr()
OFF1 = np.cumsum([0] + [m * (2 * l + 1) for m, l, _ in IRREPS_IN1]).tolist()
OFF2 = np.cumsum([0] + [m * (2 * l + 1) for m, l, _ in IRREPS_IN2]).tolist()
OUT_OFF = np.cumsum([0] + [m * (2 * l + 1) for m, l, _ in IRREPS_OUT]).tolist()
NK = len(INSTR)  # 15


def _w3j_slice(k):
    i, j, _ = INSTR[k]
    _, l1, _ = IRREPS_IN1[i]
    _, l2, _ = IRREPS_IN2[j]
    _, l3, _ = IRREPS_OUT[k]
    w3 = _w3j(l1, l2, l3)
    alpha = math.sqrt(2 * l3 + 1)

    def col(i1, k3):
        c = np.zeros(IN2_DIM, dtype=np.float64)
        c[OFF2[j]:OFF2[j] + 2 * l2 + 1] = alpha * w3[i1, :, k3]
        return c
    return col, l1, l2, l3


def _build_tables():
    """Builds bmat with two column families per instruction k:
      * narrow cols: one per nonzero (k, k3, i1), grouped by (k, k3) —
        consumed one at a time as per-partition scalars (STT chains),
        and consumed d3-at-a-time for the diag form (l2==0 / l1==0).
      * wide cols: for each used i1-layer r, a block of d3 cols
        (k3-major, zero where that (k3, i1=r) term is absent) — consumed
        as k3-varying tensor operands for whole-block TT ops.
    """
    bcols = []
    chains = {}   # (k,k3) -> [(i1, col)]
    layers = {}   # k -> [(i1, col0)] one per used i1-layer, col0 = first of d3
    for k in range(NK):
        colfn, l1, l2, l3 = _w3j_slice(k)
        d3 = 2 * l3 + 1
        used = set()
        for k3 in range(d3):
            chains[(k, k3)] = []
            for i1 in range(2 * l1 + 1):
                v = colfn(i1, k3)
                if np.max(np.abs(v)) < 1e-12:
                    continue
                used.add(i1)
                chains[(k, k3)].append((i1, len(bcols)))
                bcols.append(v)
        layers[k] = []
        for i1 in sorted(used):
            layers[k].append((i1, len(bcols)))
            for k3 in range(d3):
                bcols.append(colfn(i1, k3))
    bmat = np.stack(bcols, axis=1).astype(np.float32)
    return bmat, chains, layers


BMAT, CHAINS, LAYERS = _build_tables()
NCOL = BMAT.shape[1]


def _is_diag(k):
    """True when every (k,k3) chain is the single term i1 == a*k3 + b with
    uniform step (l1==0 -> i1=0; l2==0 -> i1=k3), so the whole block is one
    TT against d3 consecutive narrow cols."""
    _, l3, _ = IRREPS_OUT[k]
    d3 = 2 * l3 + 1
    ch = [CHAINS[(k, k3)] for k3 in range(d3)]
    if any(len(c) != 1 for c in ch):
        return None
    i1s = [c[0][0] for c in ch]
    cols = [c[0][1] for c in ch]
    if cols != list(range(cols[0], cols[0] + d3)):
        return None
    step = 0 if len(i1s) == 1 else i1s[1] - i1s[0]
    if any(i1s[t] != i1s[0] + step * t for t in range(len(i1s))):
        return None
    return (i1s[0], step, cols[0])


# exact per-op throughput costs from the CoreSim cost model (ns):
#   DVE: init 60ns + 1.042/elem (TS single-src gets 0.5x; STT/TT 1x)
#   POOL: 95ns Q7 launch + 0.833/elem / efficiency (TT .42, TS .60)
#   ACT: 185ns + 0.833/elem
def _c_v_ts(w):
    return 60 + 0.521 * w


def _c_v_tt(w):
    return 60 + 1.042 * w


def _c_g_ts(w):
    return 95 + 1.389 * w


def _c_g_tt(w):
    return 95 + 1.984 * w


def _c_a_ts(w):
    return 185 + 0.833 * w


def _optimize_assignment():
    """Assign work items to DVE(v)/POOL(g)/ACT(a) minimizing makespan.

    Items: per-chain (k, k3) ops (or one item for diag/g_wide instructions)
    plus per-k weight multiplies.  Chain forms: who does the chain's first
    term (v/a/g; rest is always DVE STT) or whole-chain on POOL.
    """
    items = {}  # id -> {form: {eng: cost}}
    for k in range(NK):
        _, l3, _ = IRREPS_OUT[k]
        d3 = 2 * l3 + 1
        W = 64 * d3
        if _is_diag(k):
            # job+wmul merged: "<jobform>|<wmul engine>", or the fused
            # per-column (x1*T)*w custom-DVE op (no separate wmul).
            jf = {"v_diag": {"v": _c_v_tt(W)}, "g_diag": {"g": _c_g_tt(W)},
                  "a_narrow": {"a": d3 * _c_a_ts(64)}}
            wf = {"v": {"v": _c_v_tt(W)}, "g": {"g": _c_g_tt(W)}}
            merged = {}
            for jn, jd in jf.items():
                for wn, wd in wf.items():
                    d = dict(jd)
                    for e, c in wd.items():
                        d[e] = d.get(e, 0.0) + c
                    merged[f"{jn}|{wn}"] = d
            merged["vw_fused"] = {"v": d3 * _c_v_tt(64)}
            items[(k, "job")] = merged
            continue
        items[(k, "wm")] = {"v": {"v": _c_v_tt(W)}, "g": {"g": _c_g_tt(W)}}
        L = len(LAYERS[k])
        items[(k, "job")] = {
            "chains": {},  # expands to per-chain items below
            "g_wide": {"g": (2 * L - 1) * _c_g_tt(W)},
        }
        for k3 in range(d3):
            r = len(CHAINS[(k, k3)])
            rest = max(0, r - 1)
            f = {
                "v": {"v": _c_v_ts(64) + rest * _c_v_tt(64)},
                "av": {"a": _c_a_ts(64), "v": rest * _c_v_tt(64)},
                "gv": {"g": _c_g_ts(64), "v": rest * _c_v_tt(64)},
                "g": {"g": _c_g_ts(64) + rest * (_c_g_ts(64) + _c_g_tt(64))},
            }
            if r == 1:
                f["a"] = f.pop("av")
            items[(k, "chain", k3)] = f

    def solve(active):
        load = {"v": 0.0, "g": 0.0, "a": 700.0}
        pick = {}
        order = sorted(active, key=lambda it: -min(
            sum(d.values()) for d in items[it].values()))
        for it in order:
            best = None
            for name, d in items[it].items():
                nl = dict(load)
                for e, c in d.items():
                    nl[e] += c
                key = (tuple(sorted(nl.values(), reverse=True)),
                       sum(d.values()))
                if best is None or key < best[0]:
                    best = (key, name, d)
            pick[it] = best[1]
            for e, c in best[2].items():
                load[e] += c
        for _ in range(400):
            improved = False
            for it in order:
                cur = items[it][pick[it]]
                base = dict(load)
                for e, c in cur.items():
                    base[e] -= c
                cur_key = tuple(sorted(load.values(), reverse=True))
                for name, d in items[it].items():
                    if name == pick[it]:
                        continue
                    nl = dict(base)
                    for e, c in d.items():
                        nl[e] += c
                    if tuple(sorted(nl.values(), reverse=True)) < cur_key:
                        load = nl
                        pick[it] = name
                        improved = True
                        break
            if not improved:
                break
        return pick, load

    # choose chains-vs-g_wide per non-diag k greedily
    wide_ks = set()
    nondiag = [k for k in range(NK) if not _is_diag(k)]

    def active_set():
        act = []
        for k in range(NK):
            _, l3, _ = IRREPS_OUT[k]
            if _is_diag(k):
                act.append((k, "job"))  # wmul merged into job options
            elif k in wide_ks:
                act.append((k, "wm"))
                act.append((k, "job"))
            else:
                act.append((k, "wm"))
                act += [(k, "chain", k3) for k3 in range(2 * l3 + 1)]
        return act

    # for wide-k items restrict to g_wide form
    for k in nondiag:
        items[(k, "job")] = {"g_wide": items[(k, "job")]["g_wide"]}

    pick, load = solve(active_set())
    best = (max(load.values()), dict(pick), dict(load), set(wide_ks))
    for k in sorted(nondiag, key=lambda k: -len(LAYERS[k])):
        wide_ks.add(k)
        pick, load = solve(active_set())
        if max(load.values()) < best[0] - 1e-9:
            best = (max(load.values()), dict(pick), dict(load), set(wide_ks))
        else:
            wide_ks.discard(k)
    _, pick, load, wide_ks = best

    forms = {}
    wmuls = {}
    for k in range(NK):
        if _is_diag(k):
            f = pick[(k, "job")]
            if f == "vw_fused":
                forms[k] = f
                wmuls[k] = None
            else:
                forms[k], wmuls[k] = f.split("|")
        elif k in wide_ks:
            forms[k] = "g_wide"
            wmuls[k] = pick[(k, "wm")]
        else:
            _, l3, _ = IRREPS_OUT[k]
            forms[k] = {k3: pick[(k, "chain", k3)]
                        for k3 in range(2 * l3 + 1)}
            wmuls[k] = pick[(k, "wm")]
    return forms, wmuls, load


K_FORM, WMUL_ENGINE, _PRED_LOAD = _optimize_assignment()


# ---------------- bass kernel ----------------

def emit(ctx: ExitStack, tc, out_ap, x1_ap, x2_ap, w_ap, bmat_ap, ident_ap,
         n_edges: int):
    import concourse.bass as bass
    from concourse import mybir

    nc = tc.nc
    f32 = mybir.dt.float32
    MULT = mybir.AluOpType.mult
    ADD = mybir.AluOpType.add

    n_tiles = (n_edges + P - 1) // P
    SB = 2  # tiles per DMA super-batch

    consts = ctx.enter_context(tc.tile_pool(name="consts", bufs=1))
    inpool = ctx.enter_context(tc.tile_pool(name="inpool", bufs=2))
    outpool = ctx.enter_context(tc.tile_pool(name="outpool", bufs=2))
    tpool = ctx.enter_context(tc.tile_pool(name="tpool", bufs=2))
    pspool = ctx.enter_context(tc.tile_pool(name="pspool", bufs=2, space="PSUM"))

    bmat_sb = consts.tile([IN2_DIM, NCOL], f32)
    nc.sync.dma_start(out=bmat_sb[:, :], in_=bmat_ap[:, :])
    ident_sb = consts.tile([P, P], f32)
    nc.sync.dma_start(out=ident_sb[:, :], in_=ident_ap[:, :])

    # DRAM views [tile, p, dim]
    x1_v = x1_ap.rearrange("(t p) d -> t p d", p=P) if n_edges % P == 0 else None

    def dram_tile(ap, dim, e0, pb):
        return bass.AP(tensor=ap.tensor, offset=ap.offset + e0 * dim,
                       ap=[[dim, pb], [1, dim]])

    for st in range(0, n_tiles, SB):
        nt = min(SB, n_tiles - st)
        e0 = st * P
        ne = min(n_edges - e0, nt * P)

        x1_sb = inpool.tile([P, SB, IN1_DIM], f32, tag="x1")
        x2_sb = inpool.tile([P, SB, IN2_DIM], f32, tag="x2")
        w_sb = inpool.tile([P, SB, W_DIM], f32, tag="w")
        out_sb = outpool.tile([P, SB, OUT_DIM], f32, tag="out")

        for (sb_t, dram, dim) in ((x1_sb, x1_ap, IN1_DIM), (x2_sb, x2_ap, IN2_DIM),
                                  (w_sb, w_ap, W_DIM)):
            if ne == nt * P:
                src = bass.AP(tensor=dram.tensor, offset=dram.offset + e0 * dim,
                              ap=[[dim, P], [P * dim, nt], [1, dim]])
                nc.sync.dma_start(out=sb_t[:, :nt, :], in_=src)
            else:
                for b in range(nt):
                    pb = min(P, ne - b * P)
                    nc.sync.dma_start(out=sb_t[:pb, b, :],
                                      in_=dram_tile(dram, dim, e0 + b * P, pb))

        for b in range(nt):
            pb = min(P, ne - b * P)
            # ---- T build on PE ----
            if True:
                x2t_ps = pspool.tile([IN2_DIM, P], f32, tag="x2t")
                nc.tensor.transpose(out=x2t_ps[:, :pb], in_=x2_sb[:pb, b, :],
                                    identity=ident_sb[:pb, :pb])
                x2t_sb = tpool.tile([IN2_DIM, P], f32, tag="x2t_sb")
                nc.scalar.copy(out=x2t_sb[:, :pb], in_=x2t_ps[:, :pb])
                t_ps = pspool.tile([P, NCOL], f32, tag="t")
                nc.tensor.matmul(t_ps[:pb, :], x2t_sb[:, :pb], bmat_sb[:, :],
                                 start=True, stop=True)
                t_sb = tpool.tile([P, NCOL], f32, tag="t_sb")
                nc.scalar.copy(out=t_sb[:pb, :], in_=t_ps[:pb, :])

            # ---- main contraction ----
            for k in range(NK):
                i_in1, _, _ = INSTR[k]
                _, l1, _ = IRREPS_IN1[i_in1]
                _, l3, _ = IRREPS_OUT[k]
                d1 = 2 * l1 + 1
                d3 = 2 * l3 + 1
                form = K_FORM[k]
                eng = nc.gpsimd if isinstance(form, str) and \
                    form.startswith("g") else nc.vector

                # [pb, 64, d1] view of this l1 block; [pb, 64, d3] of out block
                x1blk = x1_sb[:pb, b, OFF1[i_in1]:OFF1[i_in1] + 64 * d1] \
                    .rearrange("p (u i) -> p u i", i=d1)
                outblk = out_sb[:pb, b, OUT_OFF[k]:OUT_OFF[k] + 64 * d3] \
                    .rearrange("p (u i) -> p u i", i=d3)

                def twide(c0, n):
                    return t_sb[:pb, c0:c0 + n].unsqueeze(1) \
                        .broadcast_to([pb, 64, n])

                def x1bc(i1):
                    return x1blk[:, :, i1:i1 + 1].broadcast_to([pb, 64, d3])

                if isinstance(form, dict):
                    # per-chain engine forms
                    nscr = None
                    for k3 in range(d3):
                        cform = form[k3]
                        chain = CHAINS[(k, k3)]
                        dest = outblk[:, :, k3]
                        for ci, (i1, col) in enumerate(chain):
                            src = x1blk[:, :, i1]
                            tcol = t_sb[:pb, col:col + 1]
                            if ci == 0:
                                if cform in ("a", "av"):
                                    nc.scalar.activation(
                                        out=dest, in_=src,
                                        func=mybir.ActivationFunctionType.Copy,
                                        scale=tcol)
                                elif cform in ("g", "gv"):
                                    nc.gpsimd.tensor_scalar_mul(dest, src, tcol)
                                else:
                                    nc.vector.tensor_scalar_mul(dest, src, tcol)
                            elif cform == "g":
                                if nscr is None:
                                    nscr = tpool.tile([P, 64], f32, tag="nscr")
                                nc.gpsimd.tensor_scalar_mul(nscr[:pb, :], src,
                                                            tcol)
                                nc.gpsimd.tensor_tensor(dest, dest,
                                                        nscr[:pb, :], ADD)
                            else:
                                nc.vector.scalar_tensor_tensor(
                                    dest, src, tcol, dest, op0=MULT, op1=ADD)
                elif form == "vw_fused":
                    # one DVE op per column: out = (x1*Tcol)*w  (custom DVE)
                    from concourse.dve_ops import AFFINE_MUL_REDUCE
                    wcol = w_sb[:pb, b, 64 * k:64 * (k + 1)]
                    for k3 in range(d3):
                        (i1, col), = CHAINS[(k, k3)]
                        nc.vector._custom_dve(
                            AFFINE_MUL_REDUCE, out=outblk[:, :, k3],
                            in0=x1blk[:, :, i1], in1=wcol,
                            s0=t_sb[:pb, col:col + 1], s1=0.0)
                elif form.endswith("_diag"):
                    i1_0, step, c0 = _is_diag(k)
                    if step == 0:
                        src = x1bc(i1_0)
                    else:
                        src = x1blk[:, :, i1_0:i1_0 + 1 + step * (d3 - 1):step] \
                            if step != 1 else x1blk[:, :, i1_0:i1_0 + d3]
                    eng.tensor_tensor(outblk, src, twide(c0, d3), MULT)
                elif form == "a_narrow":
                    for k3 in range(d3):
                        (i1, col), = CHAINS[(k, k3)]
                        nc.scalar.activation(
                            out=outblk[:, :, k3], in_=x1blk[:, :, i1],
                            func=mybir.ActivationFunctionType.Copy,
                            scale=t_sb[:pb, col:col + 1])
                else:  # g_wide
                    assert form == "g_wide", form
                    wscr = tpool.tile([P, 64 * 5], f32, tag="wscr")
                    for r, (i1, wc0) in enumerate(LAYERS[k]):
                        if r == 0:
                            eng.tensor_tensor(outblk, x1bc(i1), twide(wc0, d3),
                                              MULT)
                        else:
                            scr = wscr[:pb, :64 * d3].rearrange(
                                "p (u i) -> p u i", i=d3)
                            eng.tensor_tensor(scr, x1bc(i1), twide(wc0, d3),
                                              MULT)
                            eng.tensor_tensor(outblk, outblk, scr, ADD)

                # weight multiply for block k (broadcast over k3)
                if WMUL_ENGINE[k] is not None:
                    weng = nc.vector if WMUL_ENGINE[k] == "v" else nc.gpsimd
                    wb = w_sb[:pb, b, 64 * k:64 * (k + 1)].unsqueeze(2) \
                        .broadcast_to([pb, 64, d3])
                    weng.tensor_tensor(outblk, outblk, wb, MULT)

        # per-tile output DMA: tile b's output streams out while tile b+1
        # computes, and the final drain is one tile deep instead of two
        for b in range(nt):
            pb = min(P, ne - b * P)
            nc.sync.dma_start(out=dram_tile(out_ap, OUT_DIM, e0 + b * P, pb),
                              in_=out_sb[:pb, b, :])


def build(n_edges: int, repeats: int = 1):
    import concourse.bacc as bacc
    import concourse.tile as tile
    from concourse import mybir

    nc = bacc.Bacc("TRN2", target_bir_lowering=False, debug=False,
                   num_devices=N_CORES)
    f32 = mybir.dt.float32
    x1 = nc.dram_tensor("input1", [n_edges, IN1_DIM], f32, kind="ExternalInput")
    x2 = nc.dram_tensor("input2", [n_edges, IN2_DIM], f32, kind="ExternalInput")
    w = nc.dram_tensor("weights", [n_edges, W_DIM], f32, kind="ExternalInput")
    bmat = nc.dram_tensor("bmat", [IN2_DIM, NCOL], f32, kind="ExternalInput")
    ident = nc.dram_tensor("ident", [P, P], f32, kind="ExternalInput")
    out = nc.dram_tensor("out", [n_edges, OUT_DIM], f32, kind="ExternalOutput")

    with tile.TileContext(nc) as tc:
        for _ in range(repeats):
            with ExitStack() as ctx:
                emit(ctx, tc, out.ap(), x1.ap(), x2.ap(), w.ap(), bmat.ap(),
                     ident.ap(), n_edges)
    nc.compile()
    return nc


_CACHED_NC = None


def make_in_maps(input1, input2, weights):
    ident = np.eye(P, dtype=np.float32)
    in_maps = []
    for c in range(N_CORES):
        s = slice(c * E_PER_CORE, (c + 1) * E_PER_CORE)
        in_maps.append({
            "input1": np.ascontiguousarray(input1[s]),
            "input2": np.ascontiguousarray(input2[s]),
            "weights": np.ascontiguousarray(weights[s]),
            "bmat": BMAT,
            "ident": ident,
        })
    return in_maps


def kernel(input1: np.ndarray, input2: np.ndarray, weights: np.ndarray,
           trace: bool = False):
    from concourse.bass_utils import run_bass_kernel_spmd

    global _CACHED_NC
    if _CACHED_NC is None:
        _CACHED_NC = build(E_PER_CORE)
    nc = _CACHED_NC

    in_maps = make_in_maps(input1, input2, weights)
    res = run_bass_kernel_spmd(nc, in_maps, core_ids=list(range(N_CORES)),
                               trace=trace)
    out = np.concatenate([r["out"] for r in res.results], axis=0)
    kernel.last_results = res
    return out


# revision 31
# speedup vs baseline: 1.0044x; 1.0044x over previous
"""Trainium2 Bass kernel for the e3nn depthwise (uvu) tensor product.

Per edge e (E=50000), channel u (64):
    out[e, u, (k,k3)] = w[e,k,u] * sum_{i1,j2} x1[e,u,(l1,i1)] * x2[e,(l2,j2)]
                        * alpha_k * w3j_k[i1,j2,k3]

Strategy (all fp32, edge dim on SBUF partitions, fully contiguous DMAs):
  1. PE builds per-edge scalars T[e, c] = sum_j x2[e,j]*bmat[j,c] for the 129
     nonzero (k,k3,i1) combinations (bmat baked on host, alpha folded in).
  2. DVE/GPSIMD compute each output column group (k,k3) as a chain of
     fused multiply-accumulate ops: acc = x1_slice * T_col (+ acc), where the
     T column is a per-partition scalar. Then one broadcast multiply by the
     per-(e,k,u) weight per instruction k.
Sharded pure data-parallel over edges across 8 NeuronCores.
"""

import math
from contextlib import ExitStack

import numpy as np

# ---------------- problem constants (hardcoded) ----------------
E_TOTAL = 50000
N_CORES = 8
E_PER_CORE = E_TOTAL // N_CORES  # 6250
P = 128  # edges per tile (SBUF partitions)

L_MAX = 2
IRREPS_IN1 = [(64, 0, 1), (64, 1, -1), (64, 2, 1)]
IRREPS_IN2 = [(1, 0, 1), (1, 1, -1), (1, 2, 1)]

IN1_DIM = 576
IN2_DIM = 9
W_DIM = 960
OUT_DIM = 3264


def _su2_cg(j1, j2, j3):
    f = math.factorial
    C = np.zeros((2 * j1 + 1, 2 * j2 + 1, 2 * j3 + 1))
    for m1 in range(-j1, j1 + 1):
        for m2 in range(-j2, j2 + 1):
            m3 = m1 + m2
            if abs(m3) > j3:
                continue
            pref = math.sqrt((2 * j3 + 1) * f(j3 + j1 - j2) * f(j3 - j1 + j2) * f(j1 + j2 - j3) / f(j1 + j2 + j3 + 1))
            pref *= math.sqrt(f(j3 + m3) * f(j3 - m3) * f(j1 - m1) * f(j1 + m1) * f(j2 - m2) * f(j2 + m2))
            s = 0.0
            for k in range(j1 + j2 - j3 + 1):
                t = [k, j1 + j2 - j3 - k, j1 - m1 - k, j2 + m2 - k, j3 - j2 + m1 + k, j3 - j1 - m2 + k]
                if min(t) < 0:
                    continue
                s += (-1) ** k / (f(t[0]) * f(t[1]) * f(t[2]) * f(t[3]) * f(t[4]) * f(t[5]))
            C[j1 + m1, j2 + m2, j3 + m3] = pref * s
    return C / math.sqrt(2 * j3 + 1)


def _q(l):
    q = np.zeros((2 * l + 1, 2 * l + 1), dtype=np.complex128)
    for m in range(-l, 0):
        q[l + m, l + abs(m)] = 1 / math.sqrt(2)
        q[l + m, l - abs(m)] = -1j / math.sqrt(2)
    q[l, l] = 1.0
    for m in range(1, l + 1):
        q[l + m, l + abs(m)] = (-1) ** m / math.sqrt(2)
        q[l + m, l - abs(m)] = 1j * (-1) ** m / math.sqrt(2)
    return (-1j) ** l * q


def _w3j(l1, l2, l3):
    C = np.einsum("ij,kl,mn,ikm->jln", _q(l1), _q(l2), np.conj(_q(l3)), _su2_cg(l1, l2, l3))
    return np.real(C).astype(np.float64)


def _build_instr():
    irreps_out, instr = [], []
    for i, (mul1, l1, p1) in enumerate(IRREPS_IN1):
        for j, (mul2, l2, p2) in enumerate(IRREPS_IN2):
            for l3 in range(abs(l1 - l2), l1 + l2 + 1):
                if l3 <= L_MAX:
                    irreps_out.append((mul1, l3, p1 * p2))
                    instr.append((i, j, len(irreps_out) - 1))
    key = lambda k: (irreps_out[k][1], -irreps_out[k][2] * (-1) ** irreps_out[k][1])
    inv = sorted(range(len(irreps_out)), key=key)
    p = {old: new for new, old in enumerate(inv)}
    irreps_sorted = [irreps_out[i] for i in inv]
    instr = sorted([(i, j, p[k]) for (i, j, k) in instr], key=lambda t: t[2])
    return irreps_sorted, instr


IRREPS_OUT, INSTR = _build_instr()
OFF1 = np.cumsum([0] + [m * (2 * l + 1) for m, l, _ in IRREPS_IN1]).tolist()
OFF2 = np.cumsum([0] + [m * (2 * l + 1) for m, l, _ in IRREPS_IN2]).tolist()
OUT_OFF = np.cumsum([0] + [m * (2 * l + 1) for m, l, _ in IRREPS_OUT]).tolist()
NK = len(INSTR)  # 15


def _w3j_slice(k):
    i, j, _ = INSTR[k]
    _, l1, _ = IRREPS_IN1[i]
    _, l2, _ = IRREPS_IN2[j]
    _, l3, _ = IRREPS_OUT[k]
    w3 = _w3j(l1, l2, l3)
    alpha = math.sqrt(2 * l3 + 1)

    def col(i1, k3):
        c = np.zeros(IN2_DIM, dtype=np.float64)
        c[OFF2[j]:OFF2[j] + 2 * l2 + 1] = alpha * w3[i1, :, k3]
        return c
    return col, l1, l2, l3


def _build_tables():
    """Builds bmat with two column families per instruction k:
      * narrow cols: one per nonzero (k, k3, i1), grouped by (k, k3) —
        consumed one at a time as per-partition scalars (STT chains),
        and consumed d3-at-a-time for the diag form (l2==0 / l1==0).
      * wide cols: for each used i1-layer r, a block of d3 cols
        (k3-major, zero where that (k3, i1=r) term is absent) — consumed
        as k3-varying tensor operands for whole-block TT ops.
    """
    bcols = []
    chains = {}   # (k,k3) -> [(i1, col)]
    layers = {}   # k -> [(i1, col0)] one per used i1-layer, col0 = first of d3
    for k in range(NK):
        colfn, l1, l2, l3 = _w3j_slice(k)
        d3 = 2 * l3 + 1
        used = set()
        for k3 in range(d3):
            chains[(k, k3)] = []
            for i1 in range(2 * l1 + 1):
                v = colfn(i1, k3)
                if np.max(np.abs(v)) < 1e-12:
                    continue
                used.add(i1)
                chains[(k, k3)].append((i1, len(bcols)))
                bcols.append(v)
        layers[k] = []
        for i1 in sorted(used):
            layers[k].append((i1, len(bcols)))
            for k3 in range(d3):
                bcols.append(colfn(i1, k3))
    bmat = np.stack(bcols, axis=1).astype(np.float32)
    return bmat, chains, layers


BMAT, CHAINS, LAYERS = _build_tables()
NCOL = BMAT.shape[1]


def _is_diag(k):
    """True when every (k,k3) chain is the single term i1 == a*k3 + b with
    uniform step (l1==0 -> i1=0; l2==0 -> i1=k3), so the whole block is one
    TT against d3 consecutive narrow cols."""
    _, l3, _ = IRREPS_OUT[k]
    d3 = 2 * l3 + 1
    ch = [CHAINS[(k, k3)] for k3 in range(d3)]
    if any(len(c) != 1 for c in ch):
        return None
    i1s = [c[0][0] for c in ch]
    cols = [c[0][1] for c in ch]
    if cols != list(range(cols[0], cols[0] + d3)):
        return None
    step = 0 if len(i1s) == 1 else i1s[1] - i1s[0]
    if any(i1s[t] != i1s[0] + step * t for t in range(len(i1s))):
        return None
    return (i1s[0], step, cols[0])


# exact per-op throughput costs from the CoreSim cost model (ns):
#   DVE: init 60ns + 1.042/elem (TS single-src gets 0.5x; STT/TT 1x)
#   POOL: 95ns Q7 launch + 0.833/elem / efficiency (TT .42, TS .60)
#   ACT: 185ns + 0.833/elem
def _c_v_ts(w):
    return 60 + 0.521 * w


def _c_v_tt(w):
    return 60 + 1.042 * w


def _c_g_ts(w):
    return 95 + 1.389 * w


def _c_g_tt(w):
    return 95 + 1.984 * w


def _c_a_ts(w):
    return 185 + 0.833 * w


def _optimize_assignment():
    """Assign work items to DVE(v)/POOL(g)/ACT(a) minimizing makespan.

    Items: per-chain (k, k3) ops (or one item for diag/g_wide instructions)
    plus per-k weight multiplies.  Chain forms: who does the chain's first
    term (v/a/g; rest is always DVE STT) or whole-chain on POOL.
    """
    items = {}  # id -> {form: {eng: cost}}
    for k in range(NK):
        _, l3, _ = IRREPS_OUT[k]
        d3 = 2 * l3 + 1
        W = 64 * d3
        if _is_diag(k):
            # job+wmul merged: "<jobform>|<wmul engine>", or the fused
            # per-column (x1*T)*w custom-DVE op (no separate wmul).
            jf = {"v_diag": {"v": _c_v_tt(W)}, "g_diag": {"g": _c_g_tt(W)},
                  "a_narrow": {"a": d3 * _c_a_ts(64)}}
            wf = {"v": {"v": _c_v_tt(W)}, "g": {"g": _c_g_tt(W)}}
            merged = {}
            for jn, jd in jf.items():
                for wn, wd in wf.items():
                    d = dict(jd)
                    for e, c in wd.items():
                        d[e] = d.get(e, 0.0) + c
                    merged[f"{jn}|{wn}"] = d
            merged["vw_fused"] = {"v": d3 * _c_v_tt(64)}
            items[(k, "job")] = merged
            continue
        items[(k, "wm")] = {"v": {"v": _c_v_tt(W)}, "g": {"g": _c_g_tt(W)}}
        L = len(LAYERS[k])
        items[(k, "job")] = {
            "chains": {},  # expands to per-chain items below
            "g_wide": {"g": (2 * L - 1) * _c_g_tt(W)},
        }
        for k3 in range(d3):
            r = len(CHAINS[(k, k3)])
            rest = max(0, r - 1)
            f = {
                "v": {"v": _c_v_ts(64) + rest * _c_v_tt(64)},
                "av": {"a": _c_a_ts(64), "v": rest * _c_v_tt(64)},
                "gv": {"g": _c_g_ts(64), "v": rest * _c_v_tt(64)},
                "g": {"g": _c_g_ts(64) + rest * (_c_g_ts(64) + _c_g_tt(64))},
            }
            if r == 1:
                f["a"] = f.pop("av")
            items[(k, "chain", k3)] = f

    def solve(active):
        load = {"v": 0.0, "g": 0.0, "a": 700.0}
        pick = {}
        order = sorted(active, key=lambda it: -min(
            sum(d.values()) for d in items[it].values()))
        for it in order:
            best = None
            for name, d in items[it].items():
                nl = dict(load)
                for e, c in d.items():
                    nl[e] += c
                key = (tuple(sorted(nl.values(), reverse=True)),
                       sum(d.values()))
                if best is None or key < best[0]:
                    best = (key, name, d)
            pick[it] = best[1]
            for e, c in best[2].items():
                load[e] += c
        for _ in range(400):
            improved = False
            for it in order:
                cur = items[it][pick[it]]
                base = dict(load)
                for e, c in cur.items():
                    base[e] -= c
                cur_key = tuple(sorted(load.values(), reverse=True))
                for name, d in items[it].items():
                    if name == pick[it]:
                        continue
                    nl = dict(base)
                    for e, c in d.items():
                        nl[e] += c
                    if tuple(sorted(nl.values(), reverse=True)) < cur_key:
                        load = nl
                        pick[it] = name
                        improved = True
                        break
            if not improved:
                break
        return pick, load

    # choose chains-vs-g_wide per non-diag k greedily
    wide_ks = set()
    nondiag = [k for k in range(NK) if not _is_diag(k)]

    def active_set():
        act = []
        for k in range(NK):
            _, l3, _ = IRREPS_OUT[k]
            if _is_diag(k):
                act.append((k, "job"))  # wmul merged into job options
            elif k in wide_ks:
                act.append((k, "wm"))
                act.append((k, "job"))
            else:
                act.append((k, "wm"))
                act += [(k, "chain", k3) for k3 in range(2 * l3 + 1)]
        return act

    # for wide-k items restrict to g_wide form
    for k in nondiag:
        items[(k, "job")] = {"g_wide": items[(k, "job")]["g_wide"]}

    pick, load = solve(active_set())
    best = (max(load.values()), dict(pick), dict(load), set(wide_ks))
    for k in sorted(nondiag, key=lambda k: -len(LAYERS[k])):
        wide_ks.add(k)
        pick, load = solve(active_set())
        if max(load.values()) < best[0] - 1e-9:
            best = (max(load.values()), dict(pick), dict(load), set(wide_ks))
        else:
            wide_ks.discard(k)
    _, pick, load, wide_ks = best

    forms = {}
    wmuls = {}
    for k in range(NK):
        if _is_diag(k):
            f = pick[(k, "job")]
            if f == "vw_fused":
                forms[k] = f
                wmuls[k] = None
            else:
                forms[k], wmuls[k] = f.split("|")
        elif k in wide_ks:
            forms[k] = "g_wide"
            wmuls[k] = pick[(k, "wm")]
        else:
            _, l3, _ = IRREPS_OUT[k]
            forms[k] = {k3: pick[(k, "chain", k3)]
                        for k3 in range(2 * l3 + 1)}
            wmuls[k] = pick[(k, "wm")]
    return forms, wmuls, load


K_FORM, WMUL_ENGINE, _PRED_LOAD = _optimize_assignment()


# ---------------- bass kernel ----------------

def emit(ctx: ExitStack, tc, out_ap, x1_ap, x2_ap, w_ap, bmat_ap, ident_ap,
         n_edges: int):
    import concourse.bass as bass
    from concourse import mybir

    nc = tc.nc
    f32 = mybir.dt.float32
    MULT = mybir.AluOpType.mult
    ADD = mybir.AluOpType.add

    n_tiles = (n_edges + P - 1) // P
    SB = 2  # tiles per DMA super-batch

    consts = ctx.enter_context(tc.tile_pool(name="consts", bufs=1))
    inpool = ctx.enter_context(tc.tile_pool(name="inpool", bufs=2))
    outpool = ctx.enter_context(tc.tile_pool(name="outpool", bufs=2))
    tpool = ctx.enter_context(tc.tile_pool(name="tpool", bufs=2))
    pspool = ctx.enter_context(tc.tile_pool(name="pspool", bufs=2, space="PSUM"))

    bmat_sb = consts.tile([IN2_DIM, NCOL], f32)
    nc.sync.dma_start(out=bmat_sb[:, :], in_=bmat_ap[:, :])
    ident_sb = consts.tile([P, P], f32)
    nc.sync.dma_start(out=ident_sb[:, :], in_=ident_ap[:, :])

    # DRAM views [tile, p, dim]
    x1_v = x1_ap.rearrange("(t p) d -> t p d", p=P) if n_edges % P == 0 else None

    def dram_tile(ap, dim, e0, pb):
        return bass.AP(tensor=ap.tensor, offset=ap.offset + e0 * dim,
                       ap=[[dim, pb], [1, dim]])

    sbs = []
    _t0 = 0
    while _t0 < n_tiles:
        _nt = 1 if _t0 == 0 else min(SB, n_tiles - _t0)
        sbs.append((_t0, _nt))
        _t0 += _nt
    for (st, nt) in sbs:
        e0 = st * P
        ne = min(n_edges - e0, nt * P)

        x1_sb = inpool.tile([P, SB, IN1_DIM], f32, tag="x1")
        x2_sb = inpool.tile([P, SB, IN2_DIM], f32, tag="x2")
        w_sb = inpool.tile([P, SB, W_DIM], f32, tag="w")
        out_sb = outpool.tile([P, SB, OUT_DIM], f32, tag="out")

        for (sb_t, dram, dim) in ((x2_sb, x2_ap, IN2_DIM), (x1_sb, x1_ap, IN1_DIM),
                                  (w_sb, w_ap, W_DIM)):
            if ne == nt * P:
                src = bass.AP(tensor=dram.tensor, offset=dram.offset + e0 * dim,
                              ap=[[dim, P], [P * dim, nt], [1, dim]])
                nc.sync.dma_start(out=sb_t[:, :nt, :], in_=src)
            else:
                for b in range(nt):
                    pb = min(P, ne - b * P)
                    nc.sync.dma_start(out=sb_t[:pb, b, :],
                                      in_=dram_tile(dram, dim, e0 + b * P, pb))

        for b in range(nt):
            pb = min(P, ne - b * P)
            # ---- T build on PE ----
            if True:
                x2t_ps = pspool.tile([IN2_DIM, P], f32, tag="x2t")
                nc.tensor.transpose(out=x2t_ps[:, :pb], in_=x2_sb[:pb, b, :],
                                    identity=ident_sb[:pb, :pb])
                x2t_sb = tpool.tile([IN2_DIM, P], f32, tag="x2t_sb")
                nc.scalar.copy(out=x2t_sb[:, :pb], in_=x2t_ps[:, :pb])
                t_ps = pspool.tile([P, NCOL], f32, tag="t")
                nc.tensor.matmul(t_ps[:pb, :], x2t_sb[:, :pb], bmat_sb[:, :],
                                 start=True, stop=True)
                t_sb = tpool.tile([P, NCOL], f32, tag="t_sb")
                nc.scalar.copy(out=t_sb[:pb, :], in_=t_ps[:pb, :])

            # ---- main contraction ----
            for k in range(NK):
                i_in1, _, _ = INSTR[k]
                _, l1, _ = IRREPS_IN1[i_in1]
                _, l3, _ = IRREPS_OUT[k]
                d1 = 2 * l1 + 1
                d3 = 2 * l3 + 1
                form = K_FORM[k]
                eng = nc.gpsimd if isinstance(form, str) and \
                    form.startswith("g") else nc.vector

                # [pb, 64, d1] view of this l1 block; [pb, 64, d3] of out block
                x1blk = x1_sb[:pb, b, OFF1[i_in1]:OFF1[i_in1] + 64 * d1] \
                    .rearrange("p (u i) -> p u i", i=d1)
                outblk = out_sb[:pb, b, OUT_OFF[k]:OUT_OFF[k] + 64 * d3] \
                    .rearrange("p (u i) -> p u i", i=d3)

                def twide(c0, n):
                    return t_sb[:pb, c0:c0 + n].unsqueeze(1) \
                        .broadcast_to([pb, 64, n])

                def x1bc(i1):
                    return x1blk[:, :, i1:i1 + 1].broadcast_to([pb, 64, d3])

                if isinstance(form, dict):
                    # per-chain engine forms
                    nscr = None
                    for k3 in range(d3):
                        cform = form[k3]
                        chain = CHAINS[(k, k3)]
                        dest = outblk[:, :, k3]
                        for ci, (i1, col) in enumerate(chain):
                            src = x1blk[:, :, i1]
                            tcol = t_sb[:pb, col:col + 1]
                            if ci == 0:
                                if cform in ("a", "av"):
                                    nc.scalar.activation(
                                        out=dest, in_=src,
                                        func=mybir.ActivationFunctionType.Copy,
                                        scale=tcol)
                                elif cform in ("g", "gv"):
                                    nc.gpsimd.tensor_scalar_mul(dest, src, tcol)
                                else:
                                    nc.vector.tensor_scalar_mul(dest, src, tcol)
                            elif cform == "g":
                                if nscr is None:
                                    nscr = tpool.tile([P, 64], f32, tag="nscr")
                                nc.gpsimd.tensor_scalar_mul(nscr[:pb, :], src,
                                                            tcol)
                                nc.gpsimd.tensor_tensor(dest, dest,
                                                        nscr[:pb, :], ADD)
                            else:
                                nc.vector.scalar_tensor_tensor(
                                    dest, src, tcol, dest, op0=MULT, op1=ADD)
                elif form == "vw_fused":
                    # one DVE op per column: out = (x1*Tcol)*w  (custom DVE)
                    from concourse.dve_ops import AFFINE_MUL_REDUCE
                    wcol = w_sb[:pb, b, 64 * k:64 * (k + 1)]
                    for k3 in range(d3):
                        (i1, col), = CHAINS[(k, k3)]
                        nc.vector._custom_dve(
                            AFFINE_MUL_REDUCE, out=outblk[:, :, k3],
                            in0=x1blk[:, :, i1], in1=wcol,
                            s0=t_sb[:pb, col:col + 1], s1=0.0)
                elif form.endswith("_diag"):
                    i1_0, step, c0 = _is_diag(k)
                    if step == 0:
                        src = x1bc(i1_0)
                    else:
                        src = x1blk[:, :, i1_0:i1_0 + 1 + step * (d3 - 1):step] \
                            if step != 1 else x1blk[:, :, i1_0:i1_0 + d3]
                    eng.tensor_tensor(outblk, src, twide(c0, d3), MULT)
                elif form == "a_narrow":
                    for k3 in range(d3):
                        (i1, col), = CHAINS[(k, k3)]
                        nc.scalar.activation(
                            out=outblk[:, :, k3], in_=x1blk[:, :, i1],
                            func=mybir.ActivationFunctionType.Copy,
                            scale=t_sb[:pb, col:col + 1])
                else:  # g_wide
                    assert form == "g_wide", form
                    wscr = tpool.tile([P, 64 * 5], f32, tag="wscr")
                    for r, (i1, wc0) in enumerate(LAYERS[k]):
                        if r == 0:
                            eng.tensor_tensor(outblk, x1bc(i1), twide(wc0, d3),
                                              MULT)
                        else:
                            scr = wscr[:pb, :64 * d3].rearrange(
                                "p (u i) -> p u i", i=d3)
                            eng.tensor_tensor(scr, x1bc(i1), twide(wc0, d3),
                                              MULT)
                            eng.tensor_tensor(outblk, outblk, scr, ADD)

                # weight multiply for block k (broadcast over k3)
                if WMUL_ENGINE[k] is not None:
                    weng = nc.vector if WMUL_ENGINE[k] == "v" else nc.gpsimd
                    wb = w_sb[:pb, b, 64 * k:64 * (k + 1)].unsqueeze(2) \
                        .broadcast_to([pb, 64, d3])
                    weng.tensor_tensor(outblk, outblk, wb, MULT)

        # per-tile output DMA: tile b's output streams out while tile b+1
        # computes, and the final drain is one tile deep instead of two
        for b in range(nt):
            pb = min(P, ne - b * P)
            nc.sync.dma_start(out=dram_tile(out_ap, OUT_DIM, e0 + b * P, pb),
                              in_=out_sb[:pb, b, :])


def build(n_edges: int, repeats: int = 1):
    import concourse.bacc as bacc
    import concourse.tile as tile
    from concourse import mybir

    nc = bacc.Bacc("TRN2", target_bir_lowering=False, debug=False,
                   num_devices=N_CORES)
    f32 = mybir.dt.float32
    x1 = nc.dram_tensor("input1", [n_edges, IN1_DIM], f32, kind="ExternalInput")
    x2 = nc.dram_tensor("input2", [n_edges, IN2_DIM], f32, kind="ExternalInput")
    w = nc.dram_tensor("weights", [n_edges, W_DIM], f32, kind="ExternalInput")
    bmat = nc.dram_tensor("bmat", [IN2_DIM, NCOL], f32, kind="ExternalInput")
    ident = nc.dram_tensor("ident", [P, P], f32, kind="ExternalInput")
    out = nc.dram_tensor("out", [n_edges, OUT_DIM], f32, kind="ExternalOutput")

    with tile.TileContext(nc) as tc:
        for _ in range(repeats):
            with ExitStack() as ctx:
                emit(ctx, tc, out.ap(), x1.ap(), x2.ap(), w.ap(), bmat.ap(),
                     ident.ap(), n_edges)
    nc.compile()
    return nc


_CACHED_NC = None


def make_in_maps(input1, input2, weights):
    ident = np.eye(P, dtype=np.float32)
    in_maps = []
    for c in range(N_CORES):
        s = slice(c * E_PER_CORE, (c + 1) * E_PER_CORE)
        in_maps.append({
            "input1": np.ascontiguousarray(input1[s]),
            "input2": np.ascontiguousarray(input2[s]),
            "weights": np.ascontiguousarray(weights[s]),
            "bmat": BMAT,
            "ident": ident,
        })
    return in_maps


def kernel(input1: np.ndarray, input2: np.ndarray, weights: np.ndarray,
           trace: bool = False):
    from concourse.bass_utils import run_bass_kernel_spmd

    global _CACHED_NC
    if _CACHED_NC is None:
        _CACHED_NC = build(E_PER_CORE)
    nc = _CACHED_NC

    in_maps = make_in_maps(input1, input2, weights)
    res = run_bass_kernel_spmd(nc, in_maps, core_ids=list(range(N_CORES)),
                               trace=trace)
    out = np.concatenate([r["out"] for r in res.results], axis=0)
    kernel.last_results = res
    return out
